# revision 21
# baseline (speedup 1.0000x reference)
"""Causal self-attention Trainium2 kernel.

Reference (full): x[B=2,S=2048,D=1024] @ W_qkv + b_qkv -> 16-head causal
attention -> @ W_out + b_out.

Sharding: 8 cores = (batch b in 0..1) x (head-group hg in 0..3, 4 heads of
hd=64 each). Each core computes a partial output projection for its 4 heads
on its batch; the host sums the 4 head-group partials per batch (f16
partials, f32 accumulate) and adds the (constant) V-bias correction
bv @ W_out and b_out.

Device pipeline per core (data path in fp16; accumulation in fp32 PSUM;
softmax denominator in fp32/f32r):
  - All weights and x are host-pretiled to the exact SBUF layout so every
    DMA is contiguous 2KB+ runs per partition with a cheap descriptor.
  - DMA issue is spread across 4 engine queues (sync/gpsimd/vector/scalar)
    so the critical tensors (x span 0, wqk) are all in flight within ~2us
    of the preamble instead of serializing on one queue.
  - The PE runs dummy warm-up matmuls on a scratch tile while the first
    DMAs land: the HAM activity monitor sees a busy PE and unthrottles
    (1.2 -> 2.4 GHz) before real work starts, and the PE never idles long
    enough mid-kernel to re-throttle.
  - The dense 100%-utilization projection matmuls (QK^T proj, V proj,
    output proj) are interleaved between the attention chunks so the PE
    always has independent work while ACT computes the exp.
  - Attention per (span, head): scores transposed ST[k,q] (partial-N
    matmuls below the diagonal), exp on ACT with 1/8 scale, triangle mask
    on diagonal blocks, PV accumulates attnT plus a denominator row via a
    ones column in V.
  - Normalization reads the PV accumulator straight out of PSUM: the
    denominator row is evicted by ACT (f32r, [1,512]), broadcast across
    64 partitions by a K=1 matmul into the scores PSUM ring,
    reciprocal'd on DVE, and multiplied against PSUM during the attnT
    eviction; odd heads staged through SBUF and DMA'd to partitions
    64..127.
  - Tail: the last span's out-projection pieces are held back to cover
    the final normalize chain's latency so the PE never goes idle (and
    HAM never re-throttles) before the last 16 output matmuls.
"""
import numpy as np
from contextlib import ExitStack

import concourse.bacc as bacc
import concourse.tile as tile
from concourse import mybir
from concourse.bass_utils import run_bass_kernel_spmd

F32 = mybir.dt.float32
F32R = mybir.dt.float32r
F16 = mybir.dt.float16

B = 2
S = 2048
D = 1024
HD = 64
HG = 4            # head-groups (cores per batch)
HPG = 4           # heads per group
CL = HPG * HD     # 256 local head cols per core
P = 128
NDC = D // P      # 8 d-chunks
NQJ = S // 512    # 4 q-spans
NKC = S // P      # 16 k-chunks

NDUM = 10         # PE warm-up dummy matmuls during the initial DMA wait
NDUM_BIG = 6      # first NDUM_BIG dummies are N=512, rest N=256

_CACHED = {}


def _build():
    if "nc" in _CACHED:
        return _CACHED["nc"]
    nc = bacc.Bacc("TRN2", target_bir_lowering=False, debug=False)

    xt_d = nc.dram_tensor("xt", [P, NQJ * NDC * 512], F16,
                          kind="ExternalInput")
    wqk_d = nc.dram_tensor("wqk", [P, 4 * NDC * P], F16, kind="ExternalInput")
    wv_d = nc.dram_tensor("wv", [P, 2 * NDC * P], F16, kind="ExternalInput")
    wout_d = nc.dram_tensor("wout", [P, 2 * D], F16, kind="ExternalInput")
    bqk_d = nc.dram_tensor("bqk", [P, 4], F32, kind="ExternalInput")
    tri_d = nc.dram_tensor("tri", [P, P], F16, kind="ExternalInput")
    ones_d = nc.dram_tensor("ones", [P, 68], F32, kind="ExternalInput")
    y_d = nc.dram_tensor("y", [S, D], F16, kind="ExternalOutput")

    with tile.TileContext(nc) as tc, ExitStack() as ctx:
        persist = ctx.enter_context(tc.tile_pool(name="persist", bufs=1))
        ptp = ctx.enter_context(tc.tile_pool(name="ptp", bufs=3))
        youtp = ctx.enter_context(tc.tile_pool(name="youtp", bufs=2))
        unp = ctx.enter_context(tc.tile_pool(name="unp", bufs=2))
        rcpp = ctx.enter_context(tc.tile_pool(name="rcpp", bufs=2))
        tmpp = ctx.enter_context(tc.tile_pool(name="tmpp", bufs=2))
        ps_sm = ctx.enter_context(tc.tile_pool(name="ps_sm", bufs=2, space="PSUM"))
        ps_st = ctx.enter_context(tc.tile_pool(name="ps_st", bufs=2, space="PSUM"))
        ps_av = ctx.enter_context(tc.tile_pool(name="ps_av", bufs=2, space="PSUM"))

        # ---- persistent tiles ----
        xT = persist.tile([P, NQJ, NDC, 512], F16, name="xT")       # 32KB/part
        qkt_sb = persist.tile([P, 4, S], F16, name="qkt_sb")        # 16KB/part
        v_sb = persist.tile([P, NKC, HPG, HD + 1], F16, name="v_sb")
        attnT = persist.tile([P, 2, S], F16, name="attnT")          # 8KB/part
        wout_sb = persist.tile([P, 2, D], F16, name="wout_sb")
        wqk_sb = persist.tile([P, NDC, 2 * CL], F16, name="wqk_sb")
        wv_sb = persist.tile([P, NDC, CL], F16, name="wv_sb")
        bqk_sb = persist.tile([P, 4], F32, name="bqk_sb")
        tri_sb = persist.tile([P, P], F16, name="tri_sb")
        ones_sb = persist.tile([P, 68], F32R, name="ones_sb")
        scratch = persist.tile([P, 512], F16, name="scratch")
        ones_row64 = ones_sb[64:65, 4:4 + HD]

        # ---- DMA plan: 4 issue queues in parallel, critical-first ----
        # x arrives pre-transposed AND pre-tiled from the host in the exact
        # SBUF layout: xT[p, qj, dc, qi] = x[qj*512+qi, dc*128+p].
        def x0_chunk(dc, eng):
            eng.dma_start(out=xT[:, 0, dc],
                          in_=xt_d.ap()[:, dc * 512:(dc + 1) * 512])

        def xspan_q(qj, quarter, eng):
            o = qj * NDC * 512 + quarter * 2 * 512
            eng.dma_start(
                out=xT[:, qj, 2 * quarter:2 * quarter + 2],
                in_=xt_d.ap()[:, o:o + 2 * 512]
                .rearrange("p (c s) -> p c s", s=512))

        def xspan(qj, eng):
            eng.dma_start(
                out=xT[:, qj],
                in_=xt_d.ap()[:, qj * NDC * 512:(qj + 1) * NDC * 512]
                .rearrange("p (c s) -> p c s", s=512))

        # The DMA semaphore pool is ~17 deep: keep the early entry count at
        # the pool size so no critical issue blocks on semaphore recycling.
        # x spans 2-3 / wout are issued later, from inside the span loop.
        # gpsimd: memsets + small constants, x0 high chunks, x1 quarters
        nc.gpsimd.memset(scratch, 0.0)
        nc.gpsimd.memset(v_sb[:, :, :, HD], 1.0)
        nc.gpsimd.dma_start(out=bqk_sb, in_=bqk_d.ap())
        nc.gpsimd.dma_start(out=ones_sb, in_=ones_d.ap().bitcast(F32R))
        for dc in range(4, 8):
            x0_chunk(dc, nc.gpsimd)
        xspan_q(1, 2, nc.gpsimd)
        xspan_q(1, 3, nc.gpsimd)

        # sync: tri, x0 low chunks, x1 quarters (y drains come later)
        nc.sync.dma_start(out=tri_sb, in_=tri_d.ap())
        for dc in range(4):
            x0_chunk(dc, nc.sync)
        xspan_q(1, 0, nc.sync)
        xspan_q(1, 1, nc.sync)

        # scalar: weights, critical-first (ACT compute starts ~15us)
        def wqk_piece(mc, eng):
            eng.dma_start(
                out=wqk_sb[:, :, mc * P:(mc + 1) * P],
                in_=wqk_d.ap()[:, mc * NDC * P:(mc + 1) * NDC * P]
                .rearrange("p (c m) -> p c m", m=P))

        def wv_piece(mh, eng):
            eng.dma_start(
                out=wv_sb[:, :, mh * P:(mh + 1) * P],
                in_=wv_d.ap()[:, mh * NDC * P:(mh + 1) * NDC * P]
                .rearrange("p (c m) -> p c m", m=P))

        wqk_piece(0, nc.scalar)
        wqk_piece(1, nc.scalar)
        wqk_piece(2, nc.scalar)
        wqk_piece(3, nc.scalar)
        wv_piece(0, nc.scalar)
        wv_piece(1, nc.scalar)

        # ---- PE warm-up: keep HAM busy while the first DMAs land ----
        for i in range(NDUM):
            n = 512 if i < NDUM_BIG else 256
            dps = ps_sm.tile([P, 512], F32, tag="sm", name=f"dum{i}")
            nc.tensor.matmul(dps[0:64, 0:n], scratch[:, 0:64],
                             scratch[:, 0:n], start=True, stop=True)

        # ---- dense (100%-util) projection pieces ----
        def qk_piece(qj, mc):
            q0 = qj * 512
            pq = ps_sm.tile([P, 512], F32, tag="sm", name=f"pq{qj}_{mc}")
            for kc in range(NDC):
                nc.tensor.matmul(
                    pq[:],
                    wqk_sb[:, kc, mc * P:(mc + 1) * P],
                    xT[:, qj, kc, :],
                    start=(kc == 0), stop=(kc == NDC - 1))
            nc.vector.tensor_scalar_add(
                qkt_sb[:, mc, q0:q0 + 512], pq[:], bqk_sb[:, mc:mc + 1])

        def v_piece(qj, si):
            sc = 4 * qj + si
            pv = ps_sm.tile([P, CL], F32, tag="sm", name=f"pv{sc}")
            for kc in range(NDC):
                nc.tensor.matmul(
                    pv[:],
                    xT[:, qj, kc, si * P:(si + 1) * P],
                    wv_sb[:, kc, :],
                    start=(kc == 0), stop=(kc == NDC - 1))
            nc.vector.tensor_copy(
                v_sb[:, sc, :, 0:HD],
                pv.rearrange("p (h d) -> p h d", h=HPG))

        def out_piece(qj, si, tail=False):
            # tail pieces run after all attention: borrow the idle ps_st
            # ring for 4 in-flight PSUM tiles and drain y per half
            sc = 4 * qj + si
            y_sb = youtp.tile([P, D], F16, tag="y", name=f"y{sc}")
            for oc in range(2):
                pool, tag = (ps_st, "st") if tail else (ps_sm, "sm")
                py = pool.tile([P, 512], F32, tag=tag,
                               name=f"py{sc}_{oc}")
                for cc in range(2):
                    nc.tensor.matmul(
                        py[:],
                        attnT[:, cc, sc * P:(sc + 1) * P],
                        wout_sb[:, cc, oc * 512:(oc + 1) * 512],
                        start=(cc == 0), stop=(cc == 1))
                # alternate engines so consecutive evictions overlap
                if oc == 0:
                    nc.vector.tensor_copy(
                        y_sb[:, oc * 512:(oc + 1) * 512], py[:])
                else:
                    nc.scalar.activation(
                        y_sb[:, oc * 512:(oc + 1) * 512], py[:],
                        mybir.ActivationFunctionType.Copy)
                if tail:
                    # split the drain issues across two queues so the
                    # final descriptor generation isn't serialized
                    deng = nc.sync if oc == 0 else nc.gpsimd
                    deng.dma_start(
                        out=y_d.ap()[sc * P:(sc + 1) * P,
                                     oc * 512:(oc + 1) * 512],
                        in_=y_sb[:, oc * 512:(oc + 1) * 512])
            if not tail:
                nc.sync.dma_start(out=y_d.ap()[sc * P:(sc + 1) * P, :],
                                  in_=y_sb)

        # QK proj for span 0 must precede its attention
        for mc in range(4):
            qk_piece(0, mc)

        # ---- main pipeline over q-spans ----
        for qj in range(NQJ):
            q0 = qj * 512
            nkc = 4 * (qj + 1)

            # V proj for this span (PV below consumes it)
            for si in range(4):
                v_piece(qj, si)

            # dense work to sprinkle between this span's attention chunks
            dq = []
            if qj + 1 < NQJ:
                dq += [(qk_piece, (qj + 1, mc)) for mc in range(4)]
            if qj >= 1:
                dq += [(out_piece, (qj - 1, si)) for si in range(4)]
            nchunks = 4 * (nkc // 2)

            def sched(done, qj=qj, nchunks=nchunks, ndq=len(dq)):
                # emission target for dense pieces after `done` chunks
                if qj == 0:
                    # qk(1) waits on x span 1 (HBM-bound until ~27us):
                    # keep it out of the FIFO until span 0 fully drains
                    return 0
                if qj == NQJ - 1:
                    # hold 3 pieces back to cover the final normalize
                    return 1 if done >= 12 else 0
                return min(ndq, done * ndq // nchunks)

            done = 0
            emitted = 0

            def scores_chunk(h, pi):
                """Scores pair -> exp -> mask; returns the probs tile."""
                mck, pok = 2 + h // 2, 64 * (h % 2)
                mcq, poq = h // 2, 64 * (h % 2)
                stp = ps_st.tile([P, 1024], F32, tag="st",
                                 name=f"st{qj}_{h}_{pi}")
                pt = ptp.tile([P, 1024], F16, tag="pt",
                              name=f"pt{qj}_{h}_{pi}")
                for half in range(2):
                    kc = 2 * pi + half
                    t = kc - 4 * qj
                    c0 = 128 * t if t > 0 else 0
                    nc.tensor.matmul(
                        stp[:, 512 * half + c0: 512 * half + 512],
                        qkt_sb[pok:pok + 64, mck, kc * P:(kc + 1) * P],
                        qkt_sb[poq:poq + 64, mcq, q0 + c0: q0 + 512],
                        start=True, stop=True)
                t0 = 2 * pi - 4 * qj
                ec0 = 128 * t0 if t0 > 0 else 0
                c1 = 128 * (t0 + 1) if t0 + 1 > 0 else 0
                if c1 > 0:
                    # diagonal pair: skip the unwritten causal gap
                    nc.scalar.activation(
                        pt[:, ec0:512], stp[:, ec0:512],
                        mybir.ActivationFunctionType.Exp, scale=0.125)
                    nc.scalar.activation(
                        pt[:, 512 + c1:1024], stp[:, 512 + c1:1024],
                        mybir.ActivationFunctionType.Exp, scale=0.125)
                else:
                    nc.scalar.activation(
                        pt[:, ec0:1024], stp[:, ec0:1024],
                        mybir.ActivationFunctionType.Exp, scale=0.125)
                for half in range(2):
                    kc = 2 * pi + half
                    t = kc - 4 * qj
                    if 0 <= t <= 3:
                        # masks on gpsimd: SBUF-only op, and it keeps the
                        # DVE queue short so normalize chains fire fast
                        off = 512 * half + 128 * t
                        nc.gpsimd.tensor_mul(
                            pt[:, off:off + 128],
                            pt[:, off:off + 128], tri_sb)
                return pt

            def pv_chunk(h, pi, pt, av):
                for half in range(2):
                    kc = 2 * pi + half
                    t = kc - 4 * qj
                    c0 = 128 * t if t > 0 else 0
                    nc.tensor.matmul(
                        av[0:HD + 1, c0:512],
                        v_sb[:, kc, h, :],
                        pt[:, 512 * half + c0: 512 * half + 512],
                        start=(kc == 0), stop=(kc == nkc - 1))

            def make_normalize(h, av):
                # fast-evict av (on the idle gpsimd so the DVE queue stays
                # short and the reciprocal fires promptly), then normalize
                # in SBUF; odd heads staged through SBUF and DMA'd to
                # partitions 64..127.  Split into two halves so the final
                # span can slot PE work between the evict and the rest.
                un = [None]

                def norm_a():
                    un[0] = unp.tile([HD + 1, 512], F32R, tag="un",
                                     name=f"un{qj}_{h}")
                    nc.vector.tensor_copy(un[0], av[0:HD + 1, :])

                def norm_b():
                    dnb = ps_sm.tile([P, 512], F32, tag="sm",
                                     name=f"dnb{qj}_{h}")
                    nc.tensor.matmul(dnb[0:HD, :], ones_row64,
                                     un[0][HD:HD + 1, :],
                                     start=True, stop=True)
                    rbs = rcpp.tile([HD, 512], F32, tag="rbs",
                                    name=f"rbs{qj}_{h}")
                    nc.vector.reciprocal_approx_fast(rbs, dnb[0:HD, :])
                    c = h // 2
                    if h % 2 == 0:
                        nc.vector.tensor_mul(
                            attnT[0:HD, c, q0:q0 + 512], un[0][0:HD, :],
                            rbs)
                    else:
                        tmp = tmpp.tile([HD, 512], F16, tag="tmp",
                                        name=f"tmp{qj}_{h}")
                        nc.vector.tensor_mul(tmp, un[0][0:HD, :], rbs)
                        nc.gpsimd.dma_start(
                            out=attnT[HD:P, c, q0:q0 + 512], in_=tmp)

                def norm():
                    norm_a()
                    norm_b()
                norm.parts = (norm_a, norm_b)
                return norm

            # Software-pipelined emission: PV for chunk k goes out after
            # the scores for chunk k+1, so the PE always has independent
            # matmuls to run while ACT computes the exp.  The previous
            # head's normalize chain is likewise deferred past the next
            # head's first scores chunk.
            pending_norm = None
            # odd heads first so their attnT partition-shift DMA hides
            for h in (1, 3, 0, 2):
                av = ps_av.tile([P, 512], F32, tag="av", name=f"av{qj}_{h}")
                prev_pt = None
                for pi in range(nkc // 2):
                    pt = scores_chunk(h, pi)
                    if prev_pt is not None:
                        pv_chunk(h, pi - 1, prev_pt, av)
                        if pi == 1 and pending_norm is not None:
                            # two chunks past the head boundary: the
                            # PSUM accumulator is no longer being written
                            pending_norm()
                            pending_norm = None
                    prev_pt = pt
                    # sprinkle dense pieces between attention chunks
                    done += 1
                    while emitted < sched(done):
                        f, a = dq[emitted]
                        f(*a)
                        emitted += 1
                pv_chunk(h, nkc // 2 - 1, prev_pt, av)
                pending_norm = make_normalize(h, av)
                # deferred bulk-DMA issues: late enough that the early
                # critical transfers (x0, wqk, x1) get the bandwidth, and
                # at program points where DMA semaphores are free again
                if qj == 0 and h == 1:
                    xspan(2, nc.sync)
                    nc.gpsimd.dma_start(
                        out=wout_sb, in_=wout_d.ap()
                        .rearrange("p (c o) -> p c o", o=D))
                elif qj == 1 and h == 1:
                    xspan(3, nc.gpsimd)

            # last head's normalize; leftover dense overlaps the chain:
            # evict first, cover the denominator chain with the held-back
            # dense pieces, then finish the chain
            pending_norm.parts[0]()
            if qj == 0:
                # bridge the HBM-bound wait for x span 1 with warm-up
                # matmuls so HAM never sees an idle window here
                for i in range(12):
                    dps = ps_sm.tile([P, 512], F32, tag="sm",
                                     name=f"dum1_{i}")
                    nc.tensor.matmul(dps[0:64, :], scratch[:, 0:64],
                                     scratch[:, :], start=True, stop=True)
            for f, a in dq[emitted:]:
                f(*a)
            pending_norm.parts[1]()
            pending_norm = None

        # output projection for the last span
        for si in range(4):
            out_piece(3, si, tail=True)

    nc.compile()
    _CACHED["nc"] = nc
    return nc


def _host_inputs(x, W_qkv, b_qkv, W_out):
    """Build the 8 per-core input maps."""
    x16 = np.asarray(x, dtype=np.float16)
    # [S, D] -> [p, qj, dc, qi] tile order matching the xT SBUF layout
    xt = [np.ascontiguousarray(
        x16[b].T.reshape(NDC, P, NQJ, 512).transpose(1, 2, 0, 3)
        .reshape(P, NQJ * NDC * 512)) for b in range(B)]
    tri = (np.arange(P)[None, :] >= np.arange(P)[:, None]).astype(np.float16)
    in_maps = []
    for b in range(B):
        for hg in range(HG):
            c0 = hg * CL
            # wqk pretiled: [p, (mc c m)] with wqk_sb[p, c, mc*128+m]
            w2 = np.concatenate([W_qkv[:, c0:c0 + CL],
                                 W_qkv[:, D + c0:D + c0 + CL]],
                                axis=1).astype(np.float16)  # [D, 2CL]
            wqk = np.ascontiguousarray(
                w2.reshape(NDC, P, 4, P).transpose(1, 2, 0, 3)
                .reshape(P, 4 * NDC * P))
            wvf = W_qkv[:, 2 * D + c0:2 * D + c0 + CL].astype(np.float16)
            wv = np.ascontiguousarray(
                wvf.reshape(NDC, P, 2, P).transpose(1, 2, 0, 3)
                .reshape(P, 2 * NDC * P))
            wo = W_out[c0:c0 + CL, :].astype(np.float16)  # [CL, D]
            wout = np.ascontiguousarray(
                wo.reshape(2, P, D).transpose(1, 0, 2).reshape(P, 2 * D))
            bqk = np.ascontiguousarray(
                np.concatenate([b_qkv[c0:c0 + CL],
                                b_qkv[D + c0:D + c0 + CL]])
                .reshape(4, P).T, dtype=np.float32)
            in_maps.append({
                "xt": xt[b], "wqk": wqk, "wv": wv, "wout": wout,
                "bqk": bqk, "tri": tri,
                "ones": np.ones((P, 68), dtype=np.float32),
            })
    return in_maps


def kernel(x, W_qkv, b_qkv, W_out, b_out):
    x = np.asarray(x, dtype=np.float32)
    W_qkv = np.asarray(W_qkv, dtype=np.float32)
    b_qkv = np.asarray(b_qkv, dtype=np.float32)
    W_out = np.asarray(W_out, dtype=np.float32)
    b_out = np.asarray(b_out, dtype=np.float32)

    nc = _build()
    in_maps = _host_inputs(x, W_qkv, b_qkv, W_out)
    core_ids = list(range(8))
    res = run_bass_kernel_spmd(nc, in_maps, core_ids)
    outs = [r["y"] for r in res.results]
    bv = b_qkv[2 * D:3 * D]
    corr = (bv @ W_out + b_out).astype(np.float32)
    y = np.empty((B, S, D), dtype=np.float32)
    for b in range(B):
        acc = outs[b * HG].astype(np.float32)
        for hg in range(1, HG):
            acc += outs[b * HG + hg].astype(np.float32)
        y[b] = acc + corr
    return y


# revision 24
# speedup vs baseline: 1.0873x; 1.0873x over previous
"""Causal self-attention Trainium2 kernel.

Reference (full): x[B=2,S=2048,D=1024] @ W_qkv + b_qkv -> 16-head causal
attention -> @ W_out + b_out.

Sharding: 8 cores = (batch b in 0..1) x (head-group hg in 0..3, 4 heads of
hd=64 each). Each core computes a partial output projection for its 4 heads
on its batch; the host sums the 4 head-group partials per batch (f16
partials, f32 accumulate) and adds the (constant) V-bias correction
bv @ W_out and b_out.

Device pipeline per core (data path in fp16; accumulation in fp32 PSUM;
softmax denominator in fp32/f32r):
  - All weights and x are host-pretiled to the exact SBUF layout so every
    DMA is contiguous 2KB+ runs per partition with a cheap descriptor.
  - DMA issue is spread across 4 engine queues (sync/gpsimd/vector/scalar)
    so the critical tensors (x span 0, wqk) are all in flight within ~2us
    of the preamble instead of serializing on one queue.
  - The PE runs dummy warm-up matmuls on a scratch tile while the first
    DMAs land: the HAM activity monitor sees a busy PE and unthrottles
    (1.2 -> 2.4 GHz) before real work starts, and the PE never idles long
    enough mid-kernel to re-throttle.
  - The dense 100%-utilization projection matmuls (QK^T proj, V proj,
    output proj) are interleaved between the attention chunks so the PE
    always has independent work while ACT computes the exp.
  - Attention per (span, head): scores transposed ST[k,q] (partial-N
    matmuls below the diagonal), exp on ACT with 1/8 scale, triangle mask
    on diagonal blocks, PV accumulates attnT plus a denominator row via a
    ones column in V.
  - Normalization reads the PV accumulator straight out of PSUM: the
    denominator row is evicted by ACT (f32r, [1,512]), broadcast across
    64 partitions by a K=1 matmul into the scores PSUM ring,
    reciprocal'd on DVE, and multiplied against PSUM during the attnT
    eviction; odd heads staged through SBUF and DMA'd to partitions
    64..127.
  - Tail: the last span's out-projection pieces are held back to cover
    the final normalize chain's latency so the PE never goes idle (and
    HAM never re-throttles) before the last 16 output matmuls.
"""
import numpy as np
from contextlib import ExitStack

import concourse.bacc as bacc
import concourse.tile as tile
from concourse import mybir
from concourse.bass_utils import run_bass_kernel_spmd

F32 = mybir.dt.float32
F32R = mybir.dt.float32r
F16 = mybir.dt.float16

B = 2
S = 2048
D = 1024
HD = 64
HG = 4            # head-groups (cores per batch)
HPG = 4           # heads per group
CL = HPG * HD     # 256 local head cols per core
P = 128
NDC = D // P      # 8 d-chunks
NQJ = S // 512    # 4 q-spans
NKC = S // P      # 16 k-chunks

NDUM = 10         # PE warm-up dummy matmuls during the initial DMA wait
NDUM_BIG = 6      # first NDUM_BIG dummies are N=512, rest N=256

_CACHED = {}


def _build():
    if "nc" in _CACHED:
        return _CACHED["nc"]
    nc = bacc.Bacc("TRN2", target_bir_lowering=False, debug=False)

    xt_d = nc.dram_tensor("xt", [P, NQJ * NDC * 512], F16,
                          kind="ExternalInput")
    wqk_d = nc.dram_tensor("wqk", [P, 4 * NDC * P], F16, kind="ExternalInput")
    wv_d = nc.dram_tensor("wv", [P, 2 * NDC * P], F16, kind="ExternalInput")
    wout_d = nc.dram_tensor("wout", [P, 2 * D], F16, kind="ExternalInput")
    bqk_d = nc.dram_tensor("bqk", [P, 4], F32, kind="ExternalInput")
    tri_d = nc.dram_tensor("tri", [P, P], F16, kind="ExternalInput")
    ones_d = nc.dram_tensor("ones", [P, 68], F32, kind="ExternalInput")
    y_d = nc.dram_tensor("y", [S, D], F16, kind="ExternalOutput")

    with tile.TileContext(nc) as tc, ExitStack() as ctx:
        persist = ctx.enter_context(tc.tile_pool(name="persist", bufs=1))
        ptp = ctx.enter_context(tc.tile_pool(name="ptp", bufs=3))
        youtp = ctx.enter_context(tc.tile_pool(name="youtp", bufs=2))
        unp = ctx.enter_context(tc.tile_pool(name="unp", bufs=2))
        rcpp = ctx.enter_context(tc.tile_pool(name="rcpp", bufs=2))
        tmpp = ctx.enter_context(tc.tile_pool(name="tmpp", bufs=2))
        ps_sm = ctx.enter_context(tc.tile_pool(name="ps_sm", bufs=2, space="PSUM"))
        ps_st = ctx.enter_context(tc.tile_pool(name="ps_st", bufs=2, space="PSUM"))
        ps_av = ctx.enter_context(tc.tile_pool(name="ps_av", bufs=2, space="PSUM"))

        # ---- persistent tiles ----
        xT = persist.tile([P, NQJ, NDC, 512], F16, name="xT")       # 32KB/part
        qkt_sb = persist.tile([P, 4, S], F16, name="qkt_sb")        # 16KB/part
        v_sb = persist.tile([P, NKC, HPG, HD + 1], F16, name="v_sb")
        attnT = persist.tile([P, 2, S], F16, name="attnT")          # 8KB/part
        wout_sb = persist.tile([P, 2, D], F16, name="wout_sb")
        wqk_sb = persist.tile([P, NDC, 2 * CL], F16, name="wqk_sb")
        wv_sb = persist.tile([P, NDC, CL], F16, name="wv_sb")
        bqk_sb = persist.tile([P, 4], F32, name="bqk_sb")
        tri_sb = persist.tile([P, P], F16, name="tri_sb")
        ones_sb = persist.tile([P, 68], F32R, name="ones_sb")
        scratch = persist.tile([P, 512], F16, name="scratch")
        ones_row64 = ones_sb[64:65, 4:4 + HD]

        # ---- DMA plan: 4 issue queues in parallel, critical-first ----
        # x arrives pre-transposed AND pre-tiled from the host in the exact
        # SBUF layout: xT[p, qj, dc, qi] = x[qj*512+qi, dc*128+p].
        def x0_chunk(dc, eng):
            eng.dma_start(out=xT[:, 0, dc],
                          in_=xt_d.ap()[:, dc * 512:(dc + 1) * 512])

        def xspan_q(qj, quarter, eng):
            o = qj * NDC * 512 + quarter * 2 * 512
            eng.dma_start(
                out=xT[:, qj, 2 * quarter:2 * quarter + 2],
                in_=xt_d.ap()[:, o:o + 2 * 512]
                .rearrange("p (c s) -> p c s", s=512))

        def xspan(qj, eng):
            eng.dma_start(
                out=xT[:, qj],
                in_=xt_d.ap()[:, qj * NDC * 512:(qj + 1) * NDC * 512]
                .rearrange("p (c s) -> p c s", s=512))

        # The DMA semaphore pool is ~17 deep: keep the early entry count at
        # the pool size so no critical issue blocks on semaphore recycling.
        # x spans 2-3 / wout are issued later, from inside the span loop.
        # gpsimd: memsets + small constants, x0 high chunks, x1 quarters
        nc.gpsimd.memset(scratch, 0.0)
        nc.gpsimd.memset(v_sb[:, :, :, HD], 1.0)
        nc.gpsimd.dma_start(out=bqk_sb, in_=bqk_d.ap())
        nc.gpsimd.dma_start(out=ones_sb, in_=ones_d.ap().bitcast(F32R))
        for dc in range(4, 8):
            x0_chunk(dc, nc.gpsimd)
        xspan_q(1, 2, nc.gpsimd)
        xspan_q(1, 3, nc.gpsimd)

        # sync: tri, x0 low chunks, x1 quarters (y drains come later)
        nc.sync.dma_start(out=tri_sb, in_=tri_d.ap())
        for dc in range(4):
            x0_chunk(dc, nc.sync)
        xspan_q(1, 0, nc.sync)
        xspan_q(1, 1, nc.sync)

        # scalar: weights, critical-first (ACT compute starts ~15us)
        def wqk_piece(mc, eng):
            eng.dma_start(
                out=wqk_sb[:, :, mc * P:(mc + 1) * P],
                in_=wqk_d.ap()[:, mc * NDC * P:(mc + 1) * NDC * P]
                .rearrange("p (c m) -> p c m", m=P))

        def wv_piece(mh, eng):
            eng.dma_start(
                out=wv_sb[:, :, mh * P:(mh + 1) * P],
                in_=wv_d.ap()[:, mh * NDC * P:(mh + 1) * NDC * P]
                .rearrange("p (c m) -> p c m", m=P))

        wqk_piece(0, nc.scalar)
        wqk_piece(1, nc.scalar)
        wqk_piece(2, nc.scalar)
        wqk_piece(3, nc.scalar)
        wv_piece(0, nc.scalar)
        wv_piece(1, nc.scalar)

        # ---- PE warm-up: keep HAM busy while the first DMAs land ----
        for i in range(NDUM):
            n = 512 if i < NDUM_BIG else 256
            dps = ps_sm.tile([P, 512], F32, tag="sm", name=f"dum{i}")
            nc.tensor.matmul(dps[0:64, 0:n], scratch[:, 0:64],
                             scratch[:, 0:n], start=True, stop=True)

        # ---- dense (100%-util) projection pieces ----
        def qk_piece(qj, mc):
            q0 = qj * 512
            pq = ps_sm.tile([P, 512], F32, tag="sm", name=f"pq{qj}_{mc}")
            for kc in range(NDC):
                nc.tensor.matmul(
                    pq[:],
                    wqk_sb[:, kc, mc * P:(mc + 1) * P],
                    xT[:, qj, kc, :],
                    start=(kc == 0), stop=(kc == NDC - 1))
            nc.vector.tensor_scalar_add(
                qkt_sb[:, mc, q0:q0 + 512], pq[:], bqk_sb[:, mc:mc + 1])

        def v_piece(qj, si):
            sc = 4 * qj + si
            pv = ps_sm.tile([P, CL], F32, tag="sm", name=f"pv{sc}")
            for kc in range(NDC):
                nc.tensor.matmul(
                    pv[:],
                    xT[:, qj, kc, si * P:(si + 1) * P],
                    wv_sb[:, kc, :],
                    start=(kc == 0), stop=(kc == NDC - 1))
            nc.vector.tensor_copy(
                v_sb[:, sc, :, 0:HD],
                pv.rearrange("p (h d) -> p h d", h=HPG))

        def out_piece(qj, si, tail=False):
            # tail pieces run after all attention: borrow the idle ps_st
            # ring for 4 in-flight PSUM tiles and drain y per half
            sc = 4 * qj + si
            y_sb = youtp.tile([P, D], F16, tag="y", name=f"y{sc}")
            for oc in range(2):
                pool, tag = (ps_st, "st") if tail else (ps_sm, "sm")
                py = pool.tile([P, 512], F32, tag=tag,
                               name=f"py{sc}_{oc}")
                for cc in range(2):
                    nc.tensor.matmul(
                        py[:],
                        attnT[:, cc, sc * P:(sc + 1) * P],
                        wout_sb[:, cc, oc * 512:(oc + 1) * 512],
                        start=(cc == 0), stop=(cc == 1))
                # alternate engines so consecutive evictions overlap
                if oc == 0:
                    nc.vector.tensor_copy(
                        y_sb[:, oc * 512:(oc + 1) * 512], py[:])
                else:
                    nc.scalar.activation(
                        y_sb[:, oc * 512:(oc + 1) * 512], py[:],
                        mybir.ActivationFunctionType.Copy)
                if tail:
                    # split the drain issues across two queues so the
                    # final descriptor generation isn't serialized
                    deng = nc.sync if oc == 0 else nc.gpsimd
                    deng.dma_start(
                        out=y_d.ap()[sc * P:(sc + 1) * P,
                                     oc * 512:(oc + 1) * 512],
                        in_=y_sb[:, oc * 512:(oc + 1) * 512])
            if not tail:
                nc.sync.dma_start(out=y_d.ap()[sc * P:(sc + 1) * P, :],
                                  in_=y_sb)

        # QK proj for span 0 must precede its attention
        for mc in range(4):
            qk_piece(0, mc)

        # ---- main pipeline over q-spans ----
        for qj in range(NQJ):
            q0 = qj * 512
            nkc = 4 * (qj + 1)

            # V proj for this span (PV below consumes it)
            for si in range(4):
                v_piece(qj, si)

            # dense work to sprinkle between this span's attention chunks
            dq = []
            if qj + 1 < NQJ:
                dq += [(qk_piece, (qj + 1, mc)) for mc in range(4)]
            if qj >= 1:
                dq += [(out_piece, (qj - 1, si)) for si in range(4)]
            nchunks = 4 * (nkc // 2)

            def sched(done, qj=qj, nchunks=nchunks, ndq=len(dq)):
                # emission target for dense pieces after `done` chunks
                if qj == 0:
                    # qk(1) waits on x span 1 (HBM-bound until ~27us):
                    # keep it out of the FIFO until span 0 fully drains
                    return 0
                if qj == NQJ - 1:
                    # hold 3 pieces back to cover the final normalize
                    return 1 if done >= 12 else 0
                return min(ndq, done * ndq // nchunks)

            done = 0
            emitted = 0

            def scores_chunk(h, pi):
                """Scores pair -> exp -> mask; returns the probs tile."""
                mck, pok = 2 + h // 2, 64 * (h % 2)
                mcq, poq = h // 2, 64 * (h % 2)
                stp = ps_st.tile([P, 1024], F32, tag="st",
                                 name=f"st{qj}_{h}_{pi}")
                pt = ptp.tile([P, 1024], F16, tag="pt",
                              name=f"pt{qj}_{h}_{pi}")
                for half in range(2):
                    kc = 2 * pi + half
                    t = kc - 4 * qj
                    c0 = 128 * t if t > 0 else 0
                    nc.tensor.matmul(
                        stp[:, 512 * half + c0: 512 * half + 512],
                        qkt_sb[pok:pok + 64, mck, kc * P:(kc + 1) * P],
                        qkt_sb[poq:poq + 64, mcq, q0 + c0: q0 + 512],
                        start=True, stop=True)
                t0 = 2 * pi - 4 * qj
                ec0 = 128 * t0 if t0 > 0 else 0
                c1 = 128 * (t0 + 1) if t0 + 1 > 0 else 0
                if c1 > 0:
                    # diagonal pair: skip the unwritten causal gap
                    nc.scalar.activation(
                        pt[:, ec0:512], stp[:, ec0:512],
                        mybir.ActivationFunctionType.Exp, scale=0.125)
                    nc.scalar.activation(
                        pt[:, 512 + c1:1024], stp[:, 512 + c1:1024],
                        mybir.ActivationFunctionType.Exp, scale=0.125)
                else:
                    nc.scalar.activation(
                        pt[:, ec0:1024], stp[:, ec0:1024],
                        mybir.ActivationFunctionType.Exp, scale=0.125)
                for half in range(2):
                    kc = 2 * pi + half
                    t = kc - 4 * qj
                    if 0 <= t <= 3:
                        off = 512 * half + 128 * t
                        nc.vector.tensor_mul(
                            pt[:, off:off + 128],
                            pt[:, off:off + 128], tri_sb)
                return pt

            def pv_chunk(h, pi, pt, av):
                for half in range(2):
                    kc = 2 * pi + half
                    t = kc - 4 * qj
                    c0 = 128 * t if t > 0 else 0
                    nc.tensor.matmul(
                        av[0:HD + 1, c0:512],
                        v_sb[:, kc, h, :],
                        pt[:, 512 * half + c0: 512 * half + 512],
                        start=(kc == 0), stop=(kc == nkc - 1))

            def make_normalize(h, av):
                # fast-evict av (on the idle gpsimd so the DVE queue stays
                # short and the reciprocal fires promptly), then normalize
                # in SBUF; odd heads staged through SBUF and DMA'd to
                # partitions 64..127.  Split into two halves so the final
                # span can slot PE work between the evict and the rest.
                un = [None]

                def norm_a():
                    un[0] = unp.tile([HD + 1, 512], F32R, tag="un",
                                     name=f"un{qj}_{h}")
                    nc.vector.tensor_copy(un[0], av[0:HD + 1, :])

                def norm_b():
                    dnb = ps_sm.tile([P, 512], F32, tag="sm",
                                     name=f"dnb{qj}_{h}")
                    nc.tensor.matmul(dnb[0:HD, :], ones_row64,
                                     un[0][HD:HD + 1, :],
                                     start=True, stop=True)
                    rbs = rcpp.tile([HD, 512], F32, tag="rbs",
                                    name=f"rbs{qj}_{h}")
                    nc.vector.reciprocal_approx_fast(rbs, dnb[0:HD, :])
                    c = h // 2
                    if h % 2 == 0:
                        nc.vector.tensor_mul(
                            attnT[0:HD, c, q0:q0 + 512], un[0][0:HD, :],
                            rbs)
                    else:
                        tmp = tmpp.tile([HD, 512], F16, tag="tmp",
                                        name=f"tmp{qj}_{h}")
                        nc.vector.tensor_mul(tmp, un[0][0:HD, :], rbs)
                        nc.gpsimd.dma_start(
                            out=attnT[HD:P, c, q0:q0 + 512], in_=tmp)

                def norm():
                    norm_a()
                    norm_b()
                norm.parts = (norm_a, norm_b)
                return norm

            # Software-pipelined emission: PV for chunk k goes out after
            # the scores for chunk k+1, so the PE always has independent
            # matmuls to run while ACT computes the exp.  The previous
            # head's normalize chain is likewise deferred past the next
            # head's first scores chunk.
            pending_norm = None
            # odd heads first so their attnT partition-shift DMA hides
            for h in (1, 3, 0, 2):
                av = ps_av.tile([P, 512], F32, tag="av", name=f"av{qj}_{h}")
                prev_pt = None
                for pi in range(nkc // 2):
                    pt = scores_chunk(h, pi)
                    if prev_pt is not None:
                        pv_chunk(h, pi - 1, prev_pt, av)
                        if pi == 1 and pending_norm is not None:
                            # two chunks past the head boundary: the
                            # PSUM accumulator is no longer being written
                            pending_norm()
                            pending_norm = None
                    prev_pt = pt
                    # sprinkle dense pieces between attention chunks
                    done += 1
                    while emitted < sched(done):
                        f, a = dq[emitted]
                        f(*a)
                        emitted += 1
                pv_chunk(h, nkc // 2 - 1, prev_pt, av)
                pending_norm = make_normalize(h, av)
                # deferred bulk-DMA issues, routed through the scalar
                # queue: the exps ahead of them delay the actual issue to
                # ~17-21us, so the early critical transfers (x0, wqk, wv,
                # x1) get the HBM bandwidth to themselves first
                if qj == 0 and h == 1:
                    xspan(2, nc.scalar)
                elif qj == 0 and h == 3:
                    nc.scalar.dma_start(
                        out=wout_sb, in_=wout_d.ap()
                        .rearrange("p (c o) -> p c o", o=D))
                elif qj == 0 and h == 0:
                    xspan(3, nc.scalar)

            # last head's normalize; leftover dense overlaps the chain:
            # evict first, cover the denominator chain with the held-back
            # dense pieces, then finish the chain
            pending_norm.parts[0]()
            if qj == 0:
                # bridge the HBM-bound wait for x span 1 with warm-up
                # matmuls so HAM never sees an idle window here
                for i in range(24):
                    dps = ps_sm.tile([P, 512], F32, tag="sm",
                                     name=f"dum1_{i}")
                    nc.tensor.matmul(dps[0:64, :], scratch[:, 0:64],
                                     scratch[:, :], start=True, stop=True)
            for f, a in dq[emitted:]:
                f(*a)
            pending_norm.parts[1]()
            pending_norm = None

        # output projection for the last span
        for si in range(4):
            out_piece(3, si, tail=True)

    nc.compile()
    _CACHED["nc"] = nc
    return nc


def _host_inputs(x, W_qkv, b_qkv, W_out):
    """Build the 8 per-core input maps."""
    x16 = np.asarray(x, dtype=np.float16)
    # [S, D] -> [p, qj, dc, qi] tile order matching the xT SBUF layout
    xt = [np.ascontiguousarray(
        x16[b].T.reshape(NDC, P, NQJ, 512).transpose(1, 2, 0, 3)
        .reshape(P, NQJ * NDC * 512)) for b in range(B)]
    tri = (np.arange(P)[None, :] >= np.arange(P)[:, None]).astype(np.float16)
    in_maps = []
    for b in range(B):
        for hg in range(HG):
            c0 = hg * CL
            # wqk pretiled: [p, (mc c m)] with wqk_sb[p, c, mc*128+m]
            w2 = np.concatenate([W_qkv[:, c0:c0 + CL],
                                 W_qkv[:, D + c0:D + c0 + CL]],
                                axis=1).astype(np.float16)  # [D, 2CL]
            wqk = np.ascontiguousarray(
                w2.reshape(NDC, P, 4, P).transpose(1, 2, 0, 3)
                .reshape(P, 4 * NDC * P))
            wvf = W_qkv[:, 2 * D + c0:2 * D + c0 + CL].astype(np.float16)
            wv = np.ascontiguousarray(
                wvf.reshape(NDC, P, 2, P).transpose(1, 2, 0, 3)
                .reshape(P, 2 * NDC * P))
            wo = W_out[c0:c0 + CL, :].astype(np.float16)  # [CL, D]
            wout = np.ascontiguousarray(
                wo.reshape(2, P, D).transpose(1, 0, 2).reshape(P, 2 * D))
            bqk = np.ascontiguousarray(
                np.concatenate([b_qkv[c0:c0 + CL],
                                b_qkv[D + c0:D + c0 + CL]])
                .reshape(4, P).T, dtype=np.float32)
            in_maps.append({
                "xt": xt[b], "wqk": wqk, "wv": wv, "wout": wout,
                "bqk": bqk, "tri": tri,
                "ones": np.ones((P, 68), dtype=np.float32),
            })
    return in_maps


def kernel(x, W_qkv, b_qkv, W_out, b_out):
    x = np.asarray(x, dtype=np.float32)
    W_qkv = np.asarray(W_qkv, dtype=np.float32)
    b_qkv = np.asarray(b_qkv, dtype=np.float32)
    W_out = np.asarray(W_out, dtype=np.float32)
    b_out = np.asarray(b_out, dtype=np.float32)

    nc = _build()
    in_maps = _host_inputs(x, W_qkv, b_qkv, W_out)
    core_ids = list(range(8))
    res = run_bass_kernel_spmd(nc, in_maps, core_ids)
    outs = [r["y"] for r in res.results]
    bv = b_qkv[2 * D:3 * D]
    corr = (bv @ W_out + b_out).astype(np.float32)
    y = np.empty((B, S, D), dtype=np.float32)
    for b in range(B):
        acc = outs[b * HG].astype(np.float32)
        for hg in range(1, HG):
            acc += outs[b * HG + hg].astype(np.float32)
        y[b] = acc + corr
    return y


# revision 25
# speedup vs baseline: 1.1189x; 1.0291x over previous
"""Causal self-attention Trainium2 kernel.

Reference (full): x[B=2,S=2048,D=1024] @ W_qkv + b_qkv -> 16-head causal
attention -> @ W_out + b_out.

Sharding: 8 cores = (batch b in 0..1) x (head-group hg in 0..3, 4 heads of
hd=64 each). Each core computes a partial output projection for its 4 heads
on its batch; the host sums the 4 head-group partials per batch (f16
partials, f32 accumulate) and adds the (constant) V-bias correction
bv @ W_out and b_out.

Device pipeline per core (data path in fp16; accumulation in fp32 PSUM;
softmax denominator in fp32/f32r):
  - All weights and x are host-pretiled to the exact SBUF layout so every
    DMA is contiguous 2KB+ runs per partition with a cheap descriptor.
  - DMA issue is spread across 4 engine queues (sync/gpsimd/vector/scalar)
    so the critical tensors (x span 0, wqk) are all in flight within ~2us
    of the preamble instead of serializing on one queue.
  - The PE runs dummy warm-up matmuls on a scratch tile while the first
    DMAs land: the HAM activity monitor sees a busy PE and unthrottles
    (1.2 -> 2.4 GHz) before real work starts, and the PE never idles long
    enough mid-kernel to re-throttle.
  - The dense 100%-utilization projection matmuls (QK^T proj, V proj,
    output proj) are interleaved between the attention chunks so the PE
    always has independent work while ACT computes the exp.
  - Attention per (span, head): scores transposed ST[k,q] (partial-N
    matmuls below the diagonal), exp on ACT with 1/8 scale, triangle mask
    on diagonal blocks, PV accumulates attnT plus a denominator row via a
    ones column in V.
  - Normalization reads the PV accumulator straight out of PSUM: the
    denominator row is evicted by ACT (f32r, [1,512]), broadcast across
    64 partitions by a K=1 matmul into the scores PSUM ring,
    reciprocal'd on DVE, and multiplied against PSUM during the attnT
    eviction; odd heads staged through SBUF and DMA'd to partitions
    64..127.
  - Tail: the last span's out-projection pieces are held back to cover
    the final normalize chain's latency so the PE never goes idle (and
    HAM never re-throttles) before the last 16 output matmuls.
"""
import numpy as np
from contextlib import ExitStack

import concourse.bacc as bacc
import concourse.tile as tile
from concourse import mybir
from concourse.bass_utils import run_bass_kernel_spmd

F32 = mybir.dt.float32
F32R = mybir.dt.float32r
F16 = mybir.dt.float16

B = 2
S = 2048
D = 1024
HD = 64
HG = 4            # head-groups (cores per batch)
HPG = 4           # heads per group
CL = HPG * HD     # 256 local head cols per core
P = 128
NDC = D // P      # 8 d-chunks
NQJ = S // 512    # 4 q-spans
NKC = S // P      # 16 k-chunks

NDUM = 10         # PE warm-up dummy matmuls during the initial DMA wait
NDUM_BIG = 6      # first NDUM_BIG dummies are N=512, rest N=256

_CACHED = {}


def _build():
    if "nc" in _CACHED:
        return _CACHED["nc"]
    nc = bacc.Bacc("TRN2", target_bir_lowering=False, debug=False)

    xt_d = nc.dram_tensor("xt", [P, NQJ * NDC * 512], F16,
                          kind="ExternalInput")
    wqk_d = nc.dram_tensor("wqk", [P, 4 * NDC * P], F16, kind="ExternalInput")
    wv_d = nc.dram_tensor("wv", [P, 2 * NDC * P], F16, kind="ExternalInput")
    wout_d = nc.dram_tensor("wout", [P, 2 * D], F16, kind="ExternalInput")
    bqk_d = nc.dram_tensor("bqk", [P, 4], F32, kind="ExternalInput")
    tri_d = nc.dram_tensor("tri", [P, P], F16, kind="ExternalInput")
    ones_d = nc.dram_tensor("ones", [P, 68], F32, kind="ExternalInput")
    y_d = nc.dram_tensor("y", [S, D], F16, kind="ExternalOutput")

    with tile.TileContext(nc) as tc, ExitStack() as ctx:
        persist = ctx.enter_context(tc.tile_pool(name="persist", bufs=1))
        ptp = ctx.enter_context(tc.tile_pool(name="ptp", bufs=3))
        youtp = ctx.enter_context(tc.tile_pool(name="youtp", bufs=2))
        unp = ctx.enter_context(tc.tile_pool(name="unp", bufs=2))
        rcpp = ctx.enter_context(tc.tile_pool(name="rcpp", bufs=2))
        tmpp = ctx.enter_context(tc.tile_pool(name="tmpp", bufs=2))
        ps_sm = ctx.enter_context(tc.tile_pool(name="ps_sm", bufs=2, space="PSUM"))
        ps_st = ctx.enter_context(tc.tile_pool(name="ps_st", bufs=2, space="PSUM"))
        ps_av = ctx.enter_context(tc.tile_pool(name="ps_av", bufs=2, space="PSUM"))

        # ---- persistent tiles ----
        xT = persist.tile([P, NQJ, NDC, 512], F16, name="xT")       # 32KB/part
        qkt_sb = persist.tile([P, 4, S], F16, name="qkt_sb")        # 16KB/part
        v_sb = persist.tile([P, NKC, HPG, HD + 1], F16, name="v_sb")
        attnT = persist.tile([P, 2, S], F16, name="attnT")          # 8KB/part
        wout_sb = persist.tile([P, 2, D], F16, name="wout_sb")
        wqk_sb = persist.tile([P, NDC, 2 * CL], F16, name="wqk_sb")
        wv_sb = persist.tile([P, NDC, CL], F16, name="wv_sb")
        bqk_sb = persist.tile([P, 4], F32, name="bqk_sb")
        tri_sb = persist.tile([P, P], F16, name="tri_sb")
        ones_sb = persist.tile([P, 68], F32R, name="ones_sb")
        scratch = persist.tile([P, 512], F16, name="scratch")
        ones_row64 = ones_sb[64:65, 4:4 + HD]

        # ---- DMA plan: 4 issue queues in parallel, critical-first ----
        # x arrives pre-transposed AND pre-tiled from the host in the exact
        # SBUF layout: xT[p, qj, dc, qi] = x[qj*512+qi, dc*128+p].
        def x0_chunk(dc, eng):
            eng.dma_start(out=xT[:, 0, dc],
                          in_=xt_d.ap()[:, dc * 512:(dc + 1) * 512])

        def xspan_q(qj, quarter, eng):
            o = qj * NDC * 512 + quarter * 2 * 512
            eng.dma_start(
                out=xT[:, qj, 2 * quarter:2 * quarter + 2],
                in_=xt_d.ap()[:, o:o + 2 * 512]
                .rearrange("p (c s) -> p c s", s=512))

        def xspan(qj, eng):
            eng.dma_start(
                out=xT[:, qj],
                in_=xt_d.ap()[:, qj * NDC * 512:(qj + 1) * NDC * 512]
                .rearrange("p (c s) -> p c s", s=512))

        # The DMA semaphore pool is ~17 deep: keep the early entry count at
        # the pool size so no critical issue blocks on semaphore recycling.
        # x spans 2-3 / wout are issued later, from inside the span loop.
        # gpsimd: memsets + small constants, x0 high chunks, x1 quarters
        nc.gpsimd.memset(scratch, 0.0)
        nc.gpsimd.memset(v_sb[:, :, :, HD], 1.0)
        nc.gpsimd.dma_start(out=bqk_sb, in_=bqk_d.ap())
        nc.gpsimd.dma_start(out=ones_sb, in_=ones_d.ap().bitcast(F32R))
        for dc in range(4, 8):
            x0_chunk(dc, nc.gpsimd)
        xspan_q(1, 2, nc.gpsimd)
        xspan_q(1, 3, nc.gpsimd)

        # sync: tri, x0 low chunks, x1 quarters (y drains come later)
        nc.sync.dma_start(out=tri_sb, in_=tri_d.ap())
        for dc in range(4):
            x0_chunk(dc, nc.sync)
        xspan_q(1, 0, nc.sync)
        xspan_q(1, 1, nc.sync)

        # scalar: weights, critical-first (ACT compute starts ~15us)
        def wqk_piece(mc, eng):
            eng.dma_start(
                out=wqk_sb[:, :, mc * P:(mc + 1) * P],
                in_=wqk_d.ap()[:, mc * NDC * P:(mc + 1) * NDC * P]
                .rearrange("p (c m) -> p c m", m=P))

        def wv_piece(mh, eng):
            eng.dma_start(
                out=wv_sb[:, :, mh * P:(mh + 1) * P],
                in_=wv_d.ap()[:, mh * NDC * P:(mh + 1) * NDC * P]
                .rearrange("p (c m) -> p c m", m=P))

        wqk_piece(0, nc.scalar)
        wqk_piece(2, nc.scalar)
        wv_piece(0, nc.scalar)
        wqk_piece(1, nc.scalar)
        wqk_piece(3, nc.scalar)
        wv_piece(1, nc.scalar)

        # ---- PE warm-up: keep HAM busy while the first DMAs land ----
        for i in range(NDUM):
            n = 512 if i < NDUM_BIG else 256
            dps = ps_sm.tile([P, 512], F32, tag="sm", name=f"dum{i}")
            nc.tensor.matmul(dps[0:64, 0:n], scratch[:, 0:64],
                             scratch[:, 0:n], start=True, stop=True)

        # ---- dense (100%-util) projection pieces ----
        def qk_piece(qj, mc):
            q0 = qj * 512
            pq = ps_sm.tile([P, 512], F32, tag="sm", name=f"pq{qj}_{mc}")
            for kc in range(NDC):
                nc.tensor.matmul(
                    pq[:],
                    wqk_sb[:, kc, mc * P:(mc + 1) * P],
                    xT[:, qj, kc, :],
                    start=(kc == 0), stop=(kc == NDC - 1))
            nc.vector.tensor_scalar_add(
                qkt_sb[:, mc, q0:q0 + 512], pq[:], bqk_sb[:, mc:mc + 1])

        def v_piece(qj, si):
            sc = 4 * qj + si
            pv = ps_sm.tile([P, CL], F32, tag="sm", name=f"pv{sc}")
            for kc in range(NDC):
                nc.tensor.matmul(
                    pv[:],
                    xT[:, qj, kc, si * P:(si + 1) * P],
                    wv_sb[:, kc, :],
                    start=(kc == 0), stop=(kc == NDC - 1))
            nc.vector.tensor_copy(
                v_sb[:, sc, :, 0:HD],
                pv.rearrange("p (h d) -> p h d", h=HPG))

        def out_piece(qj, si, tail=False):
            # tail pieces run after all attention: borrow the idle ps_st
            # ring for 4 in-flight PSUM tiles and drain y per half
            sc = 4 * qj + si
            y_sb = youtp.tile([P, D], F16, tag="y", name=f"y{sc}")
            for oc in range(2):
                pool, tag = (ps_st, "st") if tail else (ps_sm, "sm")
                py = pool.tile([P, 512], F32, tag=tag,
                               name=f"py{sc}_{oc}")
                for cc in range(2):
                    nc.tensor.matmul(
                        py[:],
                        attnT[:, cc, sc * P:(sc + 1) * P],
                        wout_sb[:, cc, oc * 512:(oc + 1) * 512],
                        start=(cc == 0), stop=(cc == 1))
                # alternate engines so consecutive evictions overlap
                if oc == 0:
                    nc.vector.tensor_copy(
                        y_sb[:, oc * 512:(oc + 1) * 512], py[:])
                else:
                    nc.scalar.activation(
                        y_sb[:, oc * 512:(oc + 1) * 512], py[:],
                        mybir.ActivationFunctionType.Copy)
                if tail:
                    # split the drain issues across two queues so the
                    # final descriptor generation isn't serialized
                    deng = nc.sync if oc == 0 else nc.gpsimd
                    deng.dma_start(
                        out=y_d.ap()[sc * P:(sc + 1) * P,
                                     oc * 512:(oc + 1) * 512],
                        in_=y_sb[:, oc * 512:(oc + 1) * 512])
            if not tail:
                nc.sync.dma_start(out=y_d.ap()[sc * P:(sc + 1) * P, :],
                                  in_=y_sb)

        # QK proj for span 0 must precede its attention; heads (1,3,..)
        # consume mc (0,2) and (1,3) respectively, so load in that order
        for mc in (0, 2, 1, 3):
            qk_piece(0, mc)

        # ---- main pipeline over q-spans ----
        for qj in range(NQJ):
            q0 = qj * 512
            nkc = 4 * (qj + 1)

            # V proj for this span (PV below consumes it)
            for si in range(4):
                v_piece(qj, si)

            # dense work to sprinkle between this span's attention chunks
            dq = []
            if qj + 1 < NQJ:
                dq += [(qk_piece, (qj + 1, mc)) for mc in range(4)]
            if qj >= 1:
                dq += [(out_piece, (qj - 1, si)) for si in range(4)]
            nchunks = 4 * (nkc // 2)

            def sched(done, qj=qj, nchunks=nchunks, ndq=len(dq)):
                # emission target for dense pieces after `done` chunks
                if qj == 0:
                    # qk(1) waits on x span 1 (HBM-bound until ~27us):
                    # keep it out of the FIFO until span 0 fully drains
                    return 0
                if qj == NQJ - 1:
                    # hold 3 pieces back to cover the final normalize
                    return 1 if done >= 12 else 0
                return min(ndq, done * ndq // nchunks)

            done = 0
            emitted = 0

            def scores_chunk(h, pi):
                """Scores pair -> exp -> mask; returns the probs tile."""
                mck, pok = 2 + h // 2, 64 * (h % 2)
                mcq, poq = h // 2, 64 * (h % 2)
                stp = ps_st.tile([P, 1024], F32, tag="st",
                                 name=f"st{qj}_{h}_{pi}")
                pt = ptp.tile([P, 1024], F16, tag="pt",
                              name=f"pt{qj}_{h}_{pi}")
                for half in range(2):
                    kc = 2 * pi + half
                    t = kc - 4 * qj
                    c0 = 128 * t if t > 0 else 0
                    nc.tensor.matmul(
                        stp[:, 512 * half + c0: 512 * half + 512],
                        qkt_sb[pok:pok + 64, mck, kc * P:(kc + 1) * P],
                        qkt_sb[poq:poq + 64, mcq, q0 + c0: q0 + 512],
                        start=True, stop=True)
                t0 = 2 * pi - 4 * qj
                ec0 = 128 * t0 if t0 > 0 else 0
                c1 = 128 * (t0 + 1) if t0 + 1 > 0 else 0
                if c1 > 0:
                    # diagonal pair: skip the unwritten causal gap
                    nc.scalar.activation(
                        pt[:, ec0:512], stp[:, ec0:512],
                        mybir.ActivationFunctionType.Exp, scale=0.125)
                    nc.scalar.activation(
                        pt[:, 512 + c1:1024], stp[:, 512 + c1:1024],
                        mybir.ActivationFunctionType.Exp, scale=0.125)
                else:
                    nc.scalar.activation(
                        pt[:, ec0:1024], stp[:, ec0:1024],
                        mybir.ActivationFunctionType.Exp, scale=0.125)
                for half in range(2):
                    kc = 2 * pi + half
                    t = kc - 4 * qj
                    if 0 <= t <= 3:
                        off = 512 * half + 128 * t
                        nc.vector.tensor_mul(
                            pt[:, off:off + 128],
                            pt[:, off:off + 128], tri_sb)
                return pt

            def pv_chunk(h, pi, pt, av):
                for half in range(2):
                    kc = 2 * pi + half
                    t = kc - 4 * qj
                    c0 = 128 * t if t > 0 else 0
                    nc.tensor.matmul(
                        av[0:HD + 1, c0:512],
                        v_sb[:, kc, h, :],
                        pt[:, 512 * half + c0: 512 * half + 512],
                        start=(kc == 0), stop=(kc == nkc - 1))

            def make_normalize(h, av):
                # fast-evict av (on the idle gpsimd so the DVE queue stays
                # short and the reciprocal fires promptly), then normalize
                # in SBUF; odd heads staged through SBUF and DMA'd to
                # partitions 64..127.  Split into two halves so the final
                # span can slot PE work between the evict and the rest.
                un = [None]

                def norm_a():
                    un[0] = unp.tile([HD + 1, 512], F32R, tag="un",
                                     name=f"un{qj}_{h}")
                    nc.vector.tensor_copy(un[0], av[0:HD + 1, :])

                def norm_b():
                    dnb = ps_sm.tile([P, 512], F32, tag="sm",
                                     name=f"dnb{qj}_{h}")
                    nc.tensor.matmul(dnb[0:HD, :], ones_row64,
                                     un[0][HD:HD + 1, :],
                                     start=True, stop=True)
                    rbs = rcpp.tile([HD, 512], F32, tag="rbs",
                                    name=f"rbs{qj}_{h}")
                    nc.vector.reciprocal_approx_fast(rbs, dnb[0:HD, :])
                    c = h // 2
                    if h % 2 == 0:
                        nc.vector.tensor_mul(
                            attnT[0:HD, c, q0:q0 + 512], un[0][0:HD, :],
                            rbs)
                    else:
                        tmp = tmpp.tile([HD, 512], F16, tag="tmp",
                                        name=f"tmp{qj}_{h}")
                        nc.vector.tensor_mul(tmp, un[0][0:HD, :], rbs)
                        nc.gpsimd.dma_start(
                            out=attnT[HD:P, c, q0:q0 + 512], in_=tmp)

                def norm():
                    norm_a()
                    norm_b()
                norm.parts = (norm_a, norm_b)
                return norm

            # Software-pipelined emission: PV for chunk k goes out after
            # the scores for chunk k+1, so the PE always has independent
            # matmuls to run while ACT computes the exp.  The previous
            # head's normalize chain is likewise deferred past the next
            # head's first scores chunk.
            pending_norm = None
            # odd heads first so their attnT partition-shift DMA hides
            for h in (1, 3, 0, 2):
                av = ps_av.tile([P, 512], F32, tag="av", name=f"av{qj}_{h}")
                prev_pt = None
                for pi in range(nkc // 2):
                    pt = scores_chunk(h, pi)
                    if prev_pt is not None:
                        pv_chunk(h, pi - 1, prev_pt, av)
                        if pi == 1 and pending_norm is not None:
                            # two chunks past the head boundary: the
                            # PSUM accumulator is no longer being written
                            pending_norm()
                            pending_norm = None
                    prev_pt = pt
                    # sprinkle dense pieces between attention chunks
                    done += 1
                    while emitted < sched(done):
                        f, a = dq[emitted]
                        f(*a)
                        emitted += 1
                pv_chunk(h, nkc // 2 - 1, prev_pt, av)
                pending_norm = make_normalize(h, av)
                # deferred bulk-DMA issues, routed through the scalar
                # queue: the exps ahead of them delay the actual issue to
                # ~17-21us, so the early critical transfers (x0, wqk, wv,
                # x1) get the HBM bandwidth to themselves first
                if qj == 0 and h == 0:
                    xspan(2, nc.scalar)
                elif qj == 1 and h == 1:
                    nc.scalar.dma_start(
                        out=wout_sb, in_=wout_d.ap()
                        .rearrange("p (c o) -> p c o", o=D))
                elif qj == 1 and h == 3:
                    xspan(3, nc.scalar)

            # last head's normalize; leftover dense overlaps the chain:
            # evict first, cover the denominator chain with the held-back
            # dense pieces, then finish the chain
            pending_norm.parts[0]()
            if qj == 0:
                # bridge the HBM-bound wait for x span 1 with warm-up
                # matmuls so HAM never sees an idle window here
                for i in range(10):
                    dps = ps_sm.tile([P, 512], F32, tag="sm",
                                     name=f"dum1_{i}")
                    nc.tensor.matmul(dps[0:64, :], scratch[:, 0:64],
                                     scratch[:, :], start=True, stop=True)
            rest = list(dq[emitted:])
            if rest:
                # one covering piece hides the un-copy, then the
                # denominator chain runs on DVE under the remaining pieces
                f, a = rest[0]
                f(*a)
                pending_norm.parts[1]()
                for f, a in rest[1:]:
                    f(*a)
            else:
                pending_norm.parts[1]()
            pending_norm = None

        # output projection for the last span
        for si in range(4):
            out_piece(3, si, tail=True)

    nc.compile()
    _CACHED["nc"] = nc
    return nc


def _host_inputs(x, W_qkv, b_qkv, W_out):
    """Build the 8 per-core input maps."""
    x16 = np.asarray(x, dtype=np.float16)
    # [S, D] -> [p, qj, dc, qi] tile order matching the xT SBUF layout
    xt = [np.ascontiguousarray(
        x16[b].T.reshape(NDC, P, NQJ, 512).transpose(1, 2, 0, 3)
        .reshape(P, NQJ * NDC * 512)) for b in range(B)]
    tri = (np.arange(P)[None, :] >= np.arange(P)[:, None]).astype(np.float16)
    in_maps = []
    for b in range(B):
        for hg in range(HG):
            c0 = hg * CL
            # wqk pretiled: [p, (mc c m)] with wqk_sb[p, c, mc*128+m]
            w2 = np.concatenate([W_qkv[:, c0:c0 + CL],
                                 W_qkv[:, D + c0:D + c0 + CL]],
                                axis=1).astype(np.float16)  # [D, 2CL]
            wqk = np.ascontiguousarray(
                w2.reshape(NDC, P, 4, P).transpose(1, 2, 0, 3)
                .reshape(P, 4 * NDC * P))
            wvf = W_qkv[:, 2 * D + c0:2 * D + c0 + CL].astype(np.float16)
            wv = np.ascontiguousarray(
                wvf.reshape(NDC, P, 2, P).transpose(1, 2, 0, 3)
                .reshape(P, 2 * NDC * P))
            wo = W_out[c0:c0 + CL, :].astype(np.float16)  # [CL, D]
            wout = np.ascontiguousarray(
                wo.reshape(2, P, D).transpose(1, 0, 2).reshape(P, 2 * D))
            bqk = np.ascontiguousarray(
                np.concatenate([b_qkv[c0:c0 + CL],
                                b_qkv[D + c0:D + c0 + CL]])
                .reshape(4, P).T, dtype=np.float32)
            in_maps.append({
                "xt": xt[b], "wqk": wqk, "wv": wv, "wout": wout,
                "bqk": bqk, "tri": tri,
                "ones": np.ones((P, 68), dtype=np.float32),
            })
    return in_maps


def kernel(x, W_qkv, b_qkv, W_out, b_out):
    x = np.asarray(x, dtype=np.float32)
    W_qkv = np.asarray(W_qkv, dtype=np.float32)
    b_qkv = np.asarray(b_qkv, dtype=np.float32)
    W_out = np.asarray(W_out, dtype=np.float32)
    b_out = np.asarray(b_out, dtype=np.float32)

    nc = _build()
    in_maps = _host_inputs(x, W_qkv, b_qkv, W_out)
    core_ids = list(range(8))
    res = run_bass_kernel_spmd(nc, in_maps, core_ids)
    outs = [r["y"] for r in res.results]
    bv = b_qkv[2 * D:3 * D]
    corr = (bv @ W_out + b_out).astype(np.float32)
    y = np.empty((B, S, D), dtype=np.float32)
    for b in range(B):
        acc = outs[b * HG].astype(np.float32)
        for hg in range(1, HG):
            acc += outs[b * HG + hg].astype(np.float32)
        y[b] = acc + corr
    return y


# revision 26
# speedup vs baseline: 1.1349x; 1.0142x over previous
"""Causal self-attention Trainium2 kernel.

Reference (full): x[B=2,S=2048,D=1024] @ W_qkv + b_qkv -> 16-head causal
attention -> @ W_out + b_out.

Sharding: 8 cores = (batch b in 0..1) x (head-group hg in 0..3, 4 heads of
hd=64 each). Each core computes a partial output projection for its 4 heads
on its batch; the host sums the 4 head-group partials per batch (f16
partials, f32 accumulate) and adds the (constant) V-bias correction
bv @ W_out and b_out.

Device pipeline per core (data path in fp16; accumulation in fp32 PSUM;
softmax denominator in fp32/f32r):
  - All weights and x are host-pretiled to the exact SBUF layout so every
    DMA is contiguous 2KB+ runs per partition with a cheap descriptor.
  - DMA issue is spread across 4 engine queues (sync/gpsimd/vector/scalar)
    so the critical tensors (x span 0, wqk) are all in flight within ~2us
    of the preamble instead of serializing on one queue.
  - The PE runs dummy warm-up matmuls on a scratch tile while the first
    DMAs land: the HAM activity monitor sees a busy PE and unthrottles
    (1.2 -> 2.4 GHz) before real work starts, and the PE never idles long
    enough mid-kernel to re-throttle.
  - The dense 100%-utilization projection matmuls (QK^T proj, V proj,
    output proj) are interleaved between the attention chunks so the PE
    always has independent work while ACT computes the exp.
  - Attention per (span, head): scores transposed ST[k,q] (partial-N
    matmuls below the diagonal), exp on ACT with 1/8 scale, triangle mask
    on diagonal blocks, PV accumulates attnT plus a denominator row via a
    ones column in V.
  - Normalization reads the PV accumulator straight out of PSUM: the
    denominator row is evicted by ACT (f32r, [1,512]), broadcast across
    64 partitions by a K=1 matmul into the scores PSUM ring,
    reciprocal'd on DVE, and multiplied against PSUM during the attnT
    eviction; odd heads staged through SBUF and DMA'd to partitions
    64..127.
  - Tail: the last span's out-projection pieces are held back to cover
    the final normalize chain's latency so the PE never goes idle (and
    HAM never re-throttles) before the last 16 output matmuls.
"""
import numpy as np
from contextlib import ExitStack

import concourse.bacc as bacc
import concourse.tile as tile
from concourse import mybir
from concourse.bass_utils import run_bass_kernel_spmd

F32 = mybir.dt.float32
F32R = mybir.dt.float32r
F16 = mybir.dt.float16

B = 2
S = 2048
D = 1024
HD = 64
HG = 4            # head-groups (cores per batch)
HPG = 4           # heads per group
CL = HPG * HD     # 256 local head cols per core
P = 128
NDC = D // P      # 8 d-chunks
NQJ = S // 512    # 4 q-spans
NKC = S // P      # 16 k-chunks

NDUM = 24         # PE warm-up dummy matmuls during the initial DMA wait
NDUM_BIG = 6      # first NDUM_BIG dummies are N=512, rest N=256

_CACHED = {}


def _build():
    if "nc" in _CACHED:
        return _CACHED["nc"]
    nc = bacc.Bacc("TRN2", target_bir_lowering=False, debug=False)

    xt_d = nc.dram_tensor("xt", [P, NQJ * NDC * 512], F16,
                          kind="ExternalInput")
    wqk_d = nc.dram_tensor("wqk", [P, 4 * NDC * P], F16, kind="ExternalInput")
    wv_d = nc.dram_tensor("wv", [P, 2 * NDC * P], F16, kind="ExternalInput")
    wout_d = nc.dram_tensor("wout", [P, 2 * D], F16, kind="ExternalInput")
    bqk_d = nc.dram_tensor("bqk", [P, 4], F32, kind="ExternalInput")
    tri_d = nc.dram_tensor("tri", [P, P], F16, kind="ExternalInput")
    ones_d = nc.dram_tensor("ones", [P, 68], F32, kind="ExternalInput")
    y_d = nc.dram_tensor("y", [S, D], F16, kind="ExternalOutput")

    with tile.TileContext(nc) as tc, ExitStack() as ctx:
        persist = ctx.enter_context(tc.tile_pool(name="persist", bufs=1))
        ptp = ctx.enter_context(tc.tile_pool(name="ptp", bufs=3))
        youtp = ctx.enter_context(tc.tile_pool(name="youtp", bufs=4))
        unp = ctx.enter_context(tc.tile_pool(name="unp", bufs=2))
        rcpp = ctx.enter_context(tc.tile_pool(name="rcpp", bufs=2))
        tmpp = ctx.enter_context(tc.tile_pool(name="tmpp", bufs=2))
        ps_sm = ctx.enter_context(tc.tile_pool(name="ps_sm", bufs=2, space="PSUM"))
        ps_st = ctx.enter_context(tc.tile_pool(name="ps_st", bufs=2, space="PSUM"))
        ps_av = ctx.enter_context(tc.tile_pool(name="ps_av", bufs=2, space="PSUM"))

        # ---- persistent tiles ----
        xT = persist.tile([P, NQJ, NDC, 512], F16, name="xT")       # 32KB/part
        qkt_sb = persist.tile([P, 4, S], F16, name="qkt_sb")        # 16KB/part
        v_sb = persist.tile([P, NKC, HPG, HD + 1], F16, name="v_sb")
        attnT = persist.tile([P, 2, S], F16, name="attnT")          # 8KB/part
        wout_sb = persist.tile([P, 2, D], F16, name="wout_sb")
        wqk_sb = persist.tile([P, NDC, 2 * CL], F16, name="wqk_sb")
        wv_sb = persist.tile([P, NDC, CL], F16, name="wv_sb")
        bqk_sb = persist.tile([P, 4], F32, name="bqk_sb")
        tri_sb = persist.tile([P, P], F16, name="tri_sb")
        ones_sb = persist.tile([P, 68], F32R, name="ones_sb")
        scratch = persist.tile([P, 512], F16, name="scratch")
        ones_row64 = ones_sb[64:65, 4:4 + HD]

        # ---- DMA plan: 4 issue queues in parallel, critical-first ----
        # x arrives pre-transposed AND pre-tiled from the host in the exact
        # SBUF layout: xT[p, qj, dc, qi] = x[qj*512+qi, dc*128+p].
        def x0_chunk(dc, eng):
            eng.dma_start(out=xT[:, 0, dc],
                          in_=xt_d.ap()[:, dc * 512:(dc + 1) * 512])

        def xspan_q(qj, quarter, eng):
            o = qj * NDC * 512 + quarter * 2 * 512
            eng.dma_start(
                out=xT[:, qj, 2 * quarter:2 * quarter + 2],
                in_=xt_d.ap()[:, o:o + 2 * 512]
                .rearrange("p (c s) -> p c s", s=512))

        def xspan(qj, eng):
            eng.dma_start(
                out=xT[:, qj],
                in_=xt_d.ap()[:, qj * NDC * 512:(qj + 1) * NDC * 512]
                .rearrange("p (c s) -> p c s", s=512))

        # The DMA semaphore pool is ~17 deep: keep the early entry count at
        # the pool size so no critical issue blocks on semaphore recycling.
        # x spans 2-3 / wout are issued later, from inside the span loop.
        # gpsimd: memsets + small constants, x0 high chunks, x1 quarters
        nc.gpsimd.memset(scratch, 0.0)
        nc.gpsimd.memset(v_sb[:, :, :, HD], 1.0)
        nc.gpsimd.dma_start(out=bqk_sb, in_=bqk_d.ap())
        nc.gpsimd.dma_start(out=ones_sb, in_=ones_d.ap().bitcast(F32R))
        for dc in range(4, 8):
            x0_chunk(dc, nc.gpsimd)
        xspan_q(1, 2, nc.gpsimd)
        xspan_q(1, 3, nc.gpsimd)

        # sync: tri, x0 low chunks, x1 quarters (y drains come later)
        nc.sync.dma_start(out=tri_sb, in_=tri_d.ap())
        for dc in range(4):
            x0_chunk(dc, nc.sync)
        xspan_q(1, 0, nc.sync)
        xspan_q(1, 1, nc.sync)

        # scalar: weights, critical-first (ACT compute starts ~15us)
        def wqk_piece(mc, eng):
            eng.dma_start(
                out=wqk_sb[:, :, mc * P:(mc + 1) * P],
                in_=wqk_d.ap()[:, mc * NDC * P:(mc + 1) * NDC * P]
                .rearrange("p (c m) -> p c m", m=P))

        def wv_piece(mh, eng):
            eng.dma_start(
                out=wv_sb[:, :, mh * P:(mh + 1) * P],
                in_=wv_d.ap()[:, mh * NDC * P:(mh + 1) * NDC * P]
                .rearrange("p (c m) -> p c m", m=P))

        wqk_piece(0, nc.scalar)
        wqk_piece(2, nc.scalar)
        wv_piece(0, nc.scalar)
        wqk_piece(1, nc.scalar)
        wqk_piece(3, nc.scalar)
        wv_piece(1, nc.scalar)

        # ---- PE warm-up: keep HAM busy while the first DMAs land ----
        for i in range(NDUM):
            n = 512 if i < NDUM_BIG else 256
            dps = ps_sm.tile([P, 512], F32, tag="sm", name=f"dum{i}")
            nc.tensor.matmul(dps[0:64, 0:n], scratch[:, 0:64],
                             scratch[:, 0:n], start=True, stop=True)

        # ---- dense (100%-util) projection pieces ----
        def qk_piece(qj, mc):
            q0 = qj * 512
            pq = ps_sm.tile([P, 512], F32, tag="sm", name=f"pq{qj}_{mc}")
            for kc in range(NDC):
                nc.tensor.matmul(
                    pq[:],
                    wqk_sb[:, kc, mc * P:(mc + 1) * P],
                    xT[:, qj, kc, :],
                    start=(kc == 0), stop=(kc == NDC - 1))
            nc.vector.tensor_scalar_add(
                qkt_sb[:, mc, q0:q0 + 512], pq[:], bqk_sb[:, mc:mc + 1])

        def v_piece(qj, si):
            sc = 4 * qj + si
            pv = ps_sm.tile([P, CL], F32, tag="sm", name=f"pv{sc}")
            for kc in range(NDC):
                nc.tensor.matmul(
                    pv[:],
                    xT[:, qj, kc, si * P:(si + 1) * P],
                    wv_sb[:, kc, :],
                    start=(kc == 0), stop=(kc == NDC - 1))
            nc.vector.tensor_copy(
                v_sb[:, sc, :, 0:HD],
                pv.rearrange("p (h d) -> p h d", h=HPG))

        def out_piece(qj, si, tail=False):
            # tail pieces run after all attention: borrow the idle ps_st
            # ring for 4 in-flight PSUM tiles and drain y per half
            sc = 4 * qj + si
            y_sb = youtp.tile([P, D], F16, tag="y", name=f"y{sc}")
            for oc in range(2):
                pool, tag = (ps_st, "st") if tail else (ps_sm, "sm")
                py = pool.tile([P, 512], F32, tag=tag,
                               name=f"py{sc}_{oc}")
                for cc in range(2):
                    nc.tensor.matmul(
                        py[:],
                        attnT[:, cc, sc * P:(sc + 1) * P],
                        wout_sb[:, cc, oc * 512:(oc + 1) * 512],
                        start=(cc == 0), stop=(cc == 1))
                # alternate engines so consecutive evictions overlap
                if oc == 0:
                    nc.vector.tensor_copy(
                        y_sb[:, oc * 512:(oc + 1) * 512], py[:])
                else:
                    nc.scalar.activation(
                        y_sb[:, oc * 512:(oc + 1) * 512], py[:],
                        mybir.ActivationFunctionType.Copy)
                if tail:
                    # split the drain issues across two queues so the
                    # final descriptor generation isn't serialized
                    deng = nc.sync if oc == 0 else nc.gpsimd
                    deng.dma_start(
                        out=y_d.ap()[sc * P:(sc + 1) * P,
                                     oc * 512:(oc + 1) * 512],
                        in_=y_sb[:, oc * 512:(oc + 1) * 512])
            if not tail:
                nc.sync.dma_start(out=y_d.ap()[sc * P:(sc + 1) * P, :],
                                  in_=y_sb)

        # QK proj for span 0 must precede its attention; heads (1,3,..)
        # consume mc (0,2) and (1,3) respectively, so load in that order
        for mc in (0, 2, 1, 3):
            qk_piece(0, mc)

        # ---- main pipeline over q-spans ----
        for qj in range(NQJ):
            q0 = qj * 512
            nkc = 4 * (qj + 1)

            # V proj for this span (PV below consumes it)
            for si in range(4):
                v_piece(qj, si)

            # dense work to sprinkle between this span's attention chunks
            dq = []
            if qj + 1 < NQJ:
                dq += [(qk_piece, (qj + 1, mc)) for mc in range(4)]
            if qj >= 1:
                dq += [(out_piece, (qj - 1, si)) for si in range(4)]
            nchunks = 4 * (nkc // 2)

            def sched(done, qj=qj, nchunks=nchunks, ndq=len(dq)):
                # emission target for dense pieces after `done` chunks
                if qj == 0:
                    # qk(1) waits on x span 1 (HBM-bound until ~27us):
                    # keep it out of the FIFO until span 0 fully drains
                    return 0
                if qj == NQJ - 1:
                    # hold 3 pieces back to cover the final normalize
                    return 1 if done >= 12 else 0
                return min(ndq, done * ndq // nchunks)

            done = 0
            emitted = 0

            def scores_chunk(h, pi):
                """Scores pair -> exp -> mask; returns the probs tile."""
                mck, pok = 2 + h // 2, 64 * (h % 2)
                mcq, poq = h // 2, 64 * (h % 2)
                stp = ps_st.tile([P, 1024], F32, tag="st",
                                 name=f"st{qj}_{h}_{pi}")
                pt = ptp.tile([P, 1024], F16, tag="pt",
                              name=f"pt{qj}_{h}_{pi}")
                for half in range(2):
                    kc = 2 * pi + half
                    t = kc - 4 * qj
                    c0 = 128 * t if t > 0 else 0
                    nc.tensor.matmul(
                        stp[:, 512 * half + c0: 512 * half + 512],
                        qkt_sb[pok:pok + 64, mck, kc * P:(kc + 1) * P],
                        qkt_sb[poq:poq + 64, mcq, q0 + c0: q0 + 512],
                        start=True, stop=True)
                t0 = 2 * pi - 4 * qj
                ec0 = 128 * t0 if t0 > 0 else 0
                c1 = 128 * (t0 + 1) if t0 + 1 > 0 else 0
                if c1 > 0:
                    # diagonal pair: skip the unwritten causal gap
                    nc.scalar.activation(
                        pt[:, ec0:512], stp[:, ec0:512],
                        mybir.ActivationFunctionType.Exp, scale=0.125)
                    nc.scalar.activation(
                        pt[:, 512 + c1:1024], stp[:, 512 + c1:1024],
                        mybir.ActivationFunctionType.Exp, scale=0.125)
                else:
                    nc.scalar.activation(
                        pt[:, ec0:1024], stp[:, ec0:1024],
                        mybir.ActivationFunctionType.Exp, scale=0.125)
                for half in range(2):
                    kc = 2 * pi + half
                    t = kc - 4 * qj
                    if 0 <= t <= 3:
                        off = 512 * half + 128 * t
                        nc.vector.tensor_mul(
                            pt[:, off:off + 128],
                            pt[:, off:off + 128], tri_sb)
                return pt

            def pv_chunk(h, pi, pt, av):
                for half in range(2):
                    kc = 2 * pi + half
                    t = kc - 4 * qj
                    c0 = 128 * t if t > 0 else 0
                    nc.tensor.matmul(
                        av[0:HD + 1, c0:512],
                        v_sb[:, kc, h, :],
                        pt[:, 512 * half + c0: 512 * half + 512],
                        start=(kc == 0), stop=(kc == nkc - 1))

            def make_normalize(h, av):
                # fast-evict av (on the idle gpsimd so the DVE queue stays
                # short and the reciprocal fires promptly), then normalize
                # in SBUF; odd heads staged through SBUF and DMA'd to
                # partitions 64..127.  Split into two halves so the final
                # span can slot PE work between the evict and the rest.
                un = [None]

                def norm_a():
                    un[0] = unp.tile([HD + 1, 512], F32R, tag="un",
                                     name=f"un{qj}_{h}")
                    nc.vector.tensor_copy(un[0], av[0:HD + 1, :])

                def norm_b():
                    dnb = ps_sm.tile([P, 512], F32, tag="sm",
                                     name=f"dnb{qj}_{h}")
                    nc.tensor.matmul(dnb[0:HD, :], ones_row64,
                                     un[0][HD:HD + 1, :],
                                     start=True, stop=True)
                    rbs = rcpp.tile([HD, 512], F32, tag="rbs",
                                    name=f"rbs{qj}_{h}")
                    nc.vector.reciprocal_approx_fast(rbs, dnb[0:HD, :])
                    c = h // 2
                    if h % 2 == 0:
                        nc.vector.tensor_mul(
                            attnT[0:HD, c, q0:q0 + 512], un[0][0:HD, :],
                            rbs)
                    else:
                        tmp = tmpp.tile([HD, 512], F16, tag="tmp",
                                        name=f"tmp{qj}_{h}")
                        nc.vector.tensor_mul(tmp, un[0][0:HD, :], rbs)
                        nc.gpsimd.dma_start(
                            out=attnT[HD:P, c, q0:q0 + 512], in_=tmp)

                def norm():
                    norm_a()
                    norm_b()
                norm.parts = (norm_a, norm_b)
                return norm

            # Software-pipelined emission: PV for chunk k goes out after
            # the scores for chunk k+1, so the PE always has independent
            # matmuls to run while ACT computes the exp.  The previous
            # head's normalize chain is likewise deferred past the next
            # head's first scores chunk.
            pending_norm = None
            # odd heads first so their attnT partition-shift DMA hides
            for h in (1, 3, 0, 2):
                av = ps_av.tile([P, 512], F32, tag="av", name=f"av{qj}_{h}")
                prev_pt = None
                for pi in range(nkc // 2):
                    pt = scores_chunk(h, pi)
                    if prev_pt is not None:
                        pv_chunk(h, pi - 1, prev_pt, av)
                        if pi == 1 and pending_norm is not None:
                            # two chunks past the head boundary: the
                            # PSUM accumulator is no longer being written
                            pending_norm()
                            pending_norm = None
                    prev_pt = pt
                    # sprinkle dense pieces between attention chunks
                    done += 1
                    while emitted < sched(done):
                        f, a = dq[emitted]
                        f(*a)
                        emitted += 1
                pv_chunk(h, nkc // 2 - 1, prev_pt, av)
                pending_norm = make_normalize(h, av)
                # deferred bulk-DMA issues, routed through the scalar
                # queue: the exps ahead of them delay the actual issue to
                # ~17-21us, so the early critical transfers (x0, wqk, wv,
                # x1) get the HBM bandwidth to themselves first
                if qj == 0 and h == 0:
                    xspan(2, nc.scalar)
                elif qj == 1 and h == 1:
                    nc.scalar.dma_start(
                        out=wout_sb, in_=wout_d.ap()
                        .rearrange("p (c o) -> p c o", o=D))
                elif qj == 1 and h == 3:
                    xspan(3, nc.scalar)

            # last head's normalize; leftover dense overlaps the chain:
            # evict first, cover the denominator chain with the held-back
            # dense pieces, then finish the chain
            pending_norm.parts[0]()
            if qj == 0:
                # bridge the HBM-bound wait for x span 1 with warm-up
                # matmuls so HAM never sees an idle window here
                for i in range(14):
                    dps = ps_sm.tile([P, 512], F32, tag="sm",
                                     name=f"dum1_{i}")
                    nc.tensor.matmul(dps[0:64, :], scratch[:, 0:64],
                                     scratch[:, :], start=True, stop=True)
            rest = list(dq[emitted:])
            if rest:
                # one covering piece hides the un-copy, then the
                # denominator chain runs on DVE under the remaining pieces
                f, a = rest[0]
                f(*a)
                pending_norm.parts[1]()
                for f, a in rest[1:]:
                    f(*a)
            else:
                pending_norm.parts[1]()
            pending_norm = None

        # output projection for the last span
        for si in range(4):
            out_piece(3, si, tail=True)

    nc.compile()
    _CACHED["nc"] = nc
    return nc


def _host_inputs(x, W_qkv, b_qkv, W_out):
    """Build the 8 per-core input maps."""
    x16 = np.asarray(x, dtype=np.float16)
    # [S, D] -> [p, qj, dc, qi] tile order matching the xT SBUF layout
    xt = [np.ascontiguousarray(
        x16[b].T.reshape(NDC, P, NQJ, 512).transpose(1, 2, 0, 3)
        .reshape(P, NQJ * NDC * 512)) for b in range(B)]
    tri = (np.arange(P)[None, :] >= np.arange(P)[:, None]).astype(np.float16)
    in_maps = []
    for b in range(B):
        for hg in range(HG):
            c0 = hg * CL
            # wqk pretiled: [p, (mc c m)] with wqk_sb[p, c, mc*128+m]
            w2 = np.concatenate([W_qkv[:, c0:c0 + CL],
                                 W_qkv[:, D + c0:D + c0 + CL]],
                                axis=1).astype(np.float16)  # [D, 2CL]
            wqk = np.ascontiguousarray(
                w2.reshape(NDC, P, 4, P).transpose(1, 2, 0, 3)
                .reshape(P, 4 * NDC * P))
            wvf = W_qkv[:, 2 * D + c0:2 * D + c0 + CL].astype(np.float16)
            wv = np.ascontiguousarray(
                wvf.reshape(NDC, P, 2, P).transpose(1, 2, 0, 3)
                .reshape(P, 2 * NDC * P))
            wo = W_out[c0:c0 + CL, :].astype(np.float16)  # [CL, D]
            wout = np.ascontiguousarray(
                wo.reshape(2, P, D).transpose(1, 0, 2).reshape(P, 2 * D))
            bqk = np.ascontiguousarray(
                np.concatenate([b_qkv[c0:c0 + CL],
                                b_qkv[D + c0:D + c0 + CL]])
                .reshape(4, P).T, dtype=np.float32)
            in_maps.append({
                "xt": xt[b], "wqk": wqk, "wv": wv, "wout": wout,
                "bqk": bqk, "tri": tri,
                "ones": np.ones((P, 68), dtype=np.float32),
            })
    return in_maps


def kernel(x, W_qkv, b_qkv, W_out, b_out):
    x = np.asarray(x, dtype=np.float32)
    W_qkv = np.asarray(W_qkv, dtype=np.float32)
    b_qkv = np.asarray(b_qkv, dtype=np.float32)
    W_out = np.asarray(W_out, dtype=np.float32)
    b_out = np.asarray(b_out, dtype=np.float32)

    nc = _build()
    in_maps = _host_inputs(x, W_qkv, b_qkv, W_out)
    core_ids = list(range(8))
    res = run_bass_kernel_spmd(nc, in_maps, core_ids)
    outs = [r["y"] for r in res.results]
    bv = b_qkv[2 * D:3 * D]
    corr = (bv @ W_out + b_out).astype(np.float32)
    y = np.empty((B, S, D), dtype=np.float32)
    for b in range(B):
        acc = outs[b * HG].astype(np.float32)
        for hg in range(1, HG):
            acc += outs[b * HG + hg].astype(np.float32)
        y[b] = acc + corr
    return y
